# revision 1
# baseline (speedup 1.0000x reference)
"""BigBird-style block-sparse attention on 8 Trainium2 NeuronCores.

Problem: B=2, H=12, S=4096, D=64, BLK=64 (64 blocks), R=3 random blocks.
All mask inputs are ones (per the generator spec), so mask arithmetic is a
no-op; rand_attn drives the gather structure and is read host-side.

Sharding: 24 (b,h) pairs -> 3 per core (data + head parallel).

Device algorithm (per pair), all in "ST" layout (keys on partitions, queries
on the free axis) so no on-device transposes are needed:
  - l = 1..62 are uniform "middle" blocks: window {l-1,l,l+1} + global
    ({0,63} minus window overlap at the edges) + 3 random blocks.
  - l = 0, 63 attend densely to all keys.
  QK matmuls produce exp-able scores in PSUM, ACT does exp (scale fused),
  PV matmuls contract keys with a ones-column appended to V so the softmax
  denominator accumulates in output row 64.  Output is the unnormalized
  ctx^T [65, 4096] per pair; the host divides by row 64 and transposes.
"""

import numpy as np

B, H, S, D = 2, 12, 4096, 64
BLK = 64
NB = S // BLK            # 64
R = 3
NPAIR = B * H            # 24
NCORE = 8
PPC = NPAIR // NCORE     # 3 pairs per core
NMID = 62                # l = 1..62
NT = 31                  # r2 cross-l tiles
SCALE = 0.125            # 1/sqrt(64)

_COMPILED = {}


def _build_host_arrays(query_layer, key_layer, value_layer, rand_attn):
    import ml_dtypes
    bf16 = ml_dtypes.bfloat16

    q = np.ascontiguousarray(query_layer, dtype=np.float32).reshape(NPAIR, S, D)
    k = np.ascontiguousarray(key_layer, dtype=np.float32).reshape(NPAIR, S, D)
    v = np.ascontiguousarray(value_layer, dtype=np.float32).reshape(NPAIR, S, D)
    r = np.ascontiguousarray(rand_attn, dtype=np.int64).reshape(NPAIR, NMID, R)

    qt = np.ascontiguousarray(q.transpose(0, 2, 1)).astype(bf16)   # [24, 64, S]
    kt = np.ascontiguousarray(k.transpose(0, 2, 1)).astype(bf16)   # [24, 64, S]

    kb = k.reshape(NPAIR, NB, BLK, D)                               # [24, 64, 64, 64]
    vb = v.reshape(NPAIR, NB, BLK, D)
    bh = np.arange(NPAIR)[:, None, None]

    gk = kb[bh, r]                       # [24, 62, 3, 64, 64] (pos, d)
    gv = vb[bh, r]

    # ktr: per middle row i, the (r0, r1) pair transposed: [24, 64(d), 62*128]
    k01 = gk[:, :, 0:2]                                  # [24, 62, 2, 64, 64]
    ktr = np.ascontiguousarray(
        k01.transpose(0, 4, 1, 2, 3).reshape(NPAIR, D, NMID * 2 * BLK)
    ).astype(bf16)

    # ktr2: r2 gathered in cross-l pairs t=(i,i+1): [24, 64(d), 31*128]
    k2 = gk[:, :, 2].reshape(NPAIR, NT, 2, BLK, D)       # [24, 31, 2, 64, 64]
    ktr2 = np.ascontiguousarray(
        k2.transpose(0, 4, 1, 2, 3).reshape(NPAIR, D, NT * 2 * BLK)
    ).astype(bf16)

    ones_col = np.ones((NPAIR, 1), np.float32)

    def with_ones(blocks128):  # [24, n, 128, 64] -> [24, 128, n*65]
        n = blocks128.shape[1]
        o = np.ones((NPAIR, n, 128, 1), np.float32)
        out = np.concatenate([blocks128, o], axis=3)      # [24, n, 128, 65]
        return np.ascontiguousarray(
            out.transpose(0, 2, 1, 3).reshape(NPAIR, 128, n * 65)
        ).astype(bf16)

    # contiguous {block0, block63} packs for the global/dense matmuls
    ktg = np.ascontiguousarray(
        np.concatenate([kb[:, 0], kb[:, NB - 1]], axis=1).transpose(0, 2, 1)
    ).astype(bf16)                                                    # [24, 64, 128]
    qb = q.reshape(NPAIR, NB, BLK, D)
    qtd = np.ascontiguousarray(
        np.concatenate([qb[:, 0], qb[:, NB - 1]], axis=1).transpose(0, 2, 1)
    ).astype(bf16)                                                    # [24, 64, 128]

    vn = with_ones(v.reshape(NPAIR, NB // 2, 128, D))                 # [24,128,32*65]
    vg = with_ones(
        np.concatenate([vb[:, 0], vb[:, NB - 1]], axis=1)[:, None]    # [24,1,128,64]
    )                                                                 # [24,128,65]
    vr = with_ones(gv[:, :, 0:2].reshape(NPAIR, NMID, 128, D))        # [24,128,62*65]
    vr2 = with_ones(gv[:, :, 2].reshape(NPAIR, NT, 128, D))           # [24,128,31*65]

    del ones_col
    return dict(qt=qt, kt=kt, ktr=ktr, ktr2=ktr2, vn=vn, vg=vg, vr=vr, vr2=vr2,
                ktg=ktg, qtd=qtd)


def _fixup_multiwait(nc, mybir):
    """Split >1-sem-wait instructions (the Tile exit drain) into single-wait
    NoOps: this walrus build's CTRL codegen has one wait slot."""
    for fn in nc.m.functions:
        for bb in fn.blocks:
            insts = list(bb.instructions)
            out = []
            for inst in insts:
                si = inst.sync_info
                if si is not None and len(si.on_wait) > 1:
                    waits = list(si.on_wait)
                    for kk, w in enumerate(waits[:-1]):
                        nop = mybir.InstNoOp(
                            name=f"{inst.name}-wsplit{kk}",
                            opcode="NoOp",
                            engine=inst.engine,
                            sync_info=mybir.SyncInfo(on_wait=[w], on_update=[]),
                        )
                        out.append(nop)
                    si.on_wait = [waits[-1]]
                    inst.sync_info = si
                out.append(inst)
            bb.instructions = out


def _group_of(l):
    return min((l - 1) // 8, 7)


def _build_program(apply_fixup=True):
    import sys
    if "/opt/trn_rl_repo" not in sys.path:
        sys.path.insert(0, "/opt/trn_rl_repo")
    import concourse.bass as bass
    import concourse.mybir as mybir
    from concourse.tile import TileContext

    f32 = mybir.dt.float32
    bf16 = mybir.dt.bfloat16
    EXP = mybir.ActivationFunctionType.Exp

    nc = bass.Bass("TRN2", target_bir_lowering=False, debug=False, num_devices=NCORE)

    d_qt = nc.dram_tensor("qt", [PPC, D, S], bf16, kind="ExternalInput").ap()
    d_kt = nc.dram_tensor("kt", [PPC, D, S], bf16, kind="ExternalInput").ap()
    d_ktr = nc.dram_tensor("ktr", [PPC, D, NMID * 128], bf16, kind="ExternalInput").ap()
    d_ktr2 = nc.dram_tensor("ktr2", [PPC, D, NT * 128], bf16, kind="ExternalInput").ap()
    d_vn = nc.dram_tensor("vn", [PPC, 128, 32 * 65], bf16, kind="ExternalInput").ap()
    d_vg = nc.dram_tensor("vg", [PPC, 128, 65], bf16, kind="ExternalInput").ap()
    d_vr = nc.dram_tensor("vr", [PPC, 128, NMID * 65], bf16, kind="ExternalInput").ap()
    d_vr2 = nc.dram_tensor("vr2", [PPC, 128, NT * 65], bf16, kind="ExternalInput").ap()
    d_ktg = nc.dram_tensor("ktg", [PPC, D, 128], bf16, kind="ExternalInput").ap()
    d_qtd = nc.dram_tensor("qtd", [PPC, D, 128], bf16, kind="ExternalInput").ap()
    d_out = nc.dram_tensor("out", [PPC, 65, S], f32, kind="ExternalOutput").ap()

    # group tables: groups of middle blocks l=1..62
    GROUPS = []
    for g in range(8):
        l0 = 1 + 8 * g
        nl = 8 if g < 7 else 6
        GROUPS.append((l0, nl))

    # window pair -> group (by first client)
    pairs_of_group = [[] for _ in range(8)]
    for m in range(32):
        c0 = max(2 * m - 1, 1)
        pairs_of_group[_group_of(c0)].append(m)

    with TileContext(nc) as tc:
        with tc.tile_pool(name="sb", bufs=2) as sb, \
             tc.tile_pool(name="ps", bufs=3, space="PSUM") as ps, \
             tc.tile_pool(name="ptp", bufs=6) as ptp, \
             tc.tile_pool(name="aux", bufs=2) as aux:

            for p in range(PPC):
                qt = sb.tile([D, S], bf16, name=f"qt{p}", tag="qt")
                kt = sb.tile([D, S], bf16, name=f"kt{p}", tag="kt")
                ktr = sb.tile([D, NMID * 128], bf16, name=f"ktr{p}", tag="ktr")
                ktr2 = sb.tile([D, NT * 128], bf16, name=f"ktr2{p}", tag="ktr2")
                vn = sb.tile([128, 32 * 65], bf16, name=f"vn{p}", tag="vn")
                vg = sb.tile([128, 65], bf16, name=f"vg{p}", tag="vg")
                vr = sb.tile([128, NMID * 65], bf16, name=f"vr{p}", tag="vr")
                vr2 = sb.tile([128, NT * 65], bf16, name=f"vr2{p}", tag="vr2")
                ktg = sb.tile([D, 128], bf16, name=f"ktg{p}", tag="ktg")
                qtd = sb.tile([D, 128], bf16, name=f"qtd{p}", tag="qtd")
                for t_, d_ in ((qt, d_qt), (kt, d_kt), (ktg, d_ktg), (qtd, d_qtd)):
                    nc.sync.dma_start(out=t_, in_=d_[p])
                for t_, d_ in ((ktr, d_ktr), (ktr2, d_ktr2)):
                    nc.gpsimd.dma_start(out=t_, in_=d_[p])
                for t_, d_ in ((vn, d_vn), (vg, d_vg), (vr, d_vr), (vr2, d_vr2)):
                    nc.scalar.dma_start(out=t_, in_=d_[p])

                def vn_chunk(c):
                    return vn[:, c * 65:(c + 1) * 65]

                # ---------------- dense blocks l = 0, 63 ----------------
                ctxd = ps.tile([128, 512], f32, name=f"ctxd{p}", tag="ctx", bufs=2)
                for half in range(4):
                    std = ps.tile([128, 1024], f32, name=f"std{p}_{half}", tag="st",
                                  bufs=3)
                    for cc in range(8):
                        c = half * 8 + cc
                        nc.tensor.matmul(
                            std[:, cc * 128:(cc + 1) * 128],
                            lhsT=kt[:, c * 128:(c + 1) * 128],
                            rhs=qtd,
                            start=True, stop=True,
                        )
                    ptd = ptp.tile([128, 1024], bf16, name=f"ptd{p}_{half}", tag="pt",
                                   bufs=6)
                    nc.scalar.activation(ptd, std, EXP, scale=SCALE)
                    for cc in range(8):
                        c = half * 8 + cc
                        nc.tensor.matmul(
                            ctxd[0:65, 0:128],
                            lhsT=vn_chunk(c),
                            rhs=ptd[:, cc * 128:(cc + 1) * 128],
                            start=(c == 0), stop=(c == 31),
                        )
                od = aux.tile([128, 512], f32, name=f"od{p}", tag="outstage")
                nc.vector.tensor_copy(od[0:65, 0:128], ctxd[0:65, 0:128])
                out_blk = d_out[p].rearrange("r (x y) -> r x y", y=BLK)
                nc.sync.dma_start(out=out_blk[:, 0::(NB - 1), :],
                                  in_=od[0:65, 0:128].rearrange("r (x y) -> r x y", y=BLK))

                # ---------------- middle groups ----------------
                deferred = [[] for _ in range(9)]  # jobs: (lhsT, rhs, c_lo, c_w)

                for g in range(8):
                    l0, nl = GROUPS[g]
                    W = nl * BLK
                    t0 = 4 * g                      # first r2 cross tile of group
                    ntl = nl // 2                   # r2 tiles in group
                    ctx = ps.tile([128, 512], f32, name=f"ctx{p}_{g}", tag="ctx",
                                  bufs=2)

                    def ccols(l, w=1):
                        return slice((l - l0) * BLK, (l - l0 + w) * BLK)

                    # --- phase A: QK + exp ---
                    # alloc A: global [0:W] + rand r0r1 [512 : 512+nl*64]
                    stA = ps.tile([128, 1024], f32, name=f"stA{p}_{g}", tag="st",
                                  bufs=3)
                    nc.tensor.matmul(stA[:, 0:W], lhsT=ktg,
                                     rhs=qt[:, l0 * BLK: l0 * BLK + W],
                                     start=True, stop=True)
                    for j in range(nl):
                        i = l0 - 1 + j
                        nc.tensor.matmul(
                            stA[:, 512 + j * BLK: 512 + (j + 1) * BLK],
                            lhsT=ktr[:, i * 128:(i + 1) * 128],
                            rhs=qt[:, (l0 + j) * BLK:(l0 + j + 1) * BLK],
                            start=True, stop=True,
                        )
                    ptA = ptp.tile([128, 1024], bf16, name=f"ptA{p}_{g}", tag="pt",
                                   bufs=6)
                    if W == 512:
                        nc.scalar.activation(ptA[:, 0:512 + W], stA[:, 0:512 + W],
                                             EXP, scale=SCALE)
                    else:
                        nc.scalar.activation(ptA[:, 0:W], stA[:, 0:W], EXP,
                                             scale=SCALE)
                        nc.scalar.activation(ptA[:, 512:512 + W], stA[:, 512:512 + W],
                                             EXP, scale=SCALE)

                    # alloc B: r2 cross tiles [0 : ntl*128] + first <=2 window pairs
                    # alloc C: remaining window pairs
                    my_pairs = pairs_of_group[g]
                    stB = ps.tile([128, 1024], f32, name=f"stB{p}_{g}", tag="st",
                                  bufs=3)
                    for tt in range(ntl):
                        t = t0 + tt
                        nc.tensor.matmul(
                            stB[:, tt * 128:(tt + 1) * 128],
                            lhsT=ktr2[:, t * 128:(t + 1) * 128],
                            rhs=qt[:, (2 * t + 1) * BLK:(2 * t + 3) * BLK],
                            start=True, stop=True,
                        )
                    # window pair placement: list of (m, alloc, off, c0, c1)
                    def bank_align(off, wq):
                        if off // 512 != (off + wq - 1) // 512:
                            return ((off + 511) // 512) * 512
                        return off

                    wplace = []
                    boff = ntl * 128
                    coff = 0
                    stC = None
                    for m in my_pairs:
                        c0 = max(2 * m - 1, 1)
                        c1 = min(2 * m + 2, NMID)
                        wq = (c1 - c0 + 1) * BLK
                        b2 = bank_align(boff, wq)
                        if b2 + wq <= 1024:
                            wplace.append((m, "B", b2, c0, c1))
                            boff = b2 + wq
                        else:
                            if stC is None:
                                stC = ps.tile([128, 1024], f32, name=f"stC{p}_{g}",
                                              tag="st", bufs=3)
                            c2 = bank_align(coff, wq)
                            wplace.append((m, "C", c2, c0, c1))
                            coff = c2 + wq
                    for m, al, off, c0, c1 in wplace:
                        st = stB if al == "B" else stC
                        nc.tensor.matmul(
                            st[:, off: off + (c1 - c0 + 1) * BLK],
                            lhsT=kt[:, 2 * m * BLK:(2 * m + 2) * BLK],
                            rhs=qt[:, c0 * BLK:(c1 + 1) * BLK],
                            start=True, stop=True,
                        )
                    def merged_intervals(ivs):
                        ivs = sorted(ivs)
                        out = []
                        for a, b in ivs:
                            if out and a <= out[-1][1]:
                                out[-1][1] = max(out[-1][1], b)
                            else:
                                out.append([a, b])
                        return out

                    b_ivs = [(0, ntl * 128)] + [(off, off + (c1 - c0 + 1) * BLK)
                                                for m, al, off, c0, c1 in wplace
                                                if al == "B"]
                    c_ivs = [(off, off + (c1 - c0 + 1) * BLK)
                             for m, al, off, c0, c1 in wplace if al == "C"]
                    ptB = ptp.tile([128, 1024], bf16, name=f"ptB{p}_{g}", tag="pt",
                                   bufs=6)
                    for a, b in merged_intervals(b_ivs):
                        nc.scalar.activation(ptB[:, a:b], stB[:, a:b], EXP,
                                             scale=SCALE)
                    ptC = None
                    if stC is not None:
                        ptC = ptp.tile([128, 1024], bf16, name=f"ptC{p}_{g}",
                                       tag="pt", bufs=6)
                        for a, b in merged_intervals(c_ivs):
                            nc.scalar.activation(ptC[:, a:b], stC[:, a:b], EXP,
                                                 scale=SCALE)

                    # --- phase B: PV accumulation into ctx (ordered) ---
                    # zero dead PT regions so every PV matmul can use the
                    # full K=128 contraction (no partition-offset operands,
                    # which this device's PE cannot execute).
                    if g == 0:
                        # l=1 does not attend key block 0 via global (it is in
                        # its window instead)
                        nc.gpsimd.memset(ptA[0:64, 0:BLK], 0.0)
                    if g == 7:
                        # l=62 does not attend key block 63 via global
                        nc.gpsimd.memset(ptA[64:128, (62 - l0) * BLK:
                                             (63 - l0) * BLK], 0.0)
                    for tt in range(ntl):
                        # r2 cross tiles: only the diagonal quadrants are real
                        nc.gpsimd.memset(ptB[0:64, tt * 128 + 64:
                                             tt * 128 + 128], 0.0)
                        nc.gpsimd.memset(ptB[64:128, tt * 128:
                                             tt * 128 + 64], 0.0)
                    for m, al, off, c0, c1 in wplace:
                        pt = ptB if al == "B" else ptC
                        if 2 * m + 2 <= NMID and c1 == 2 * m + 2:
                            # key 2m (rows 0:64) not attended by l=2m+2
                            nc.gpsimd.memset(
                                pt[0:64, off + (2 * m + 2 - c0) * BLK:
                                   off + (2 * m + 3 - c0) * BLK], 0.0)
                        if m >= 1 and c0 == 2 * m - 1:
                            # key 2m+1 (rows 64:128) not attended by l=2m-1
                            nc.gpsimd.memset(pt[64:128, off: off + BLK], 0.0)
                    jobs = []
                    # global: both blocks {0,63}, full K=128, spans all W cols
                    jobs.append((vg, ptA[:, 0:W], l0, nl))
                    # deferred window parts from the previous group
                    jobs.extend(deferred[g])
                    # window pairs of this group (full K=128 per pair)
                    for m, al, off, c0, c1 in wplace:
                        pt = ptB if al == "B" else ptC
                        for gg in (g, g + 1):
                            if gg > 7:
                                continue
                            g_lo, g_nl = GROUPS[gg]
                            s_lo = max(c0, g_lo)
                            s_hi = min(c1, g_lo + g_nl - 1)
                            if s_lo > s_hi:
                                continue
                            rhs = pt[:, off + (s_lo - c0) * BLK:
                                     off + (s_hi - c0 + 1) * BLK]
                            job = (vn_chunk(m), rhs, s_lo, s_hi - s_lo + 1)
                            if gg == g:
                                jobs.append(job)
                            else:
                                deferred[gg].append(job)
                    # rand r0r1
                    for j in range(nl):
                        i = l0 - 1 + j
                        jobs.append((vr[:, i * 65:(i + 1) * 65],
                                     ptA[:, 512 + j * BLK: 512 + (j + 1) * BLK],
                                     l0 + j, 1))
                    # rand r2 cross tiles (full K=128, 2 query blocks wide)
                    for tt in range(ntl):
                        t = t0 + tt
                        jobs.append((vr2[:, t * 65:(t + 1) * 65],
                                     ptB[:, tt * 128:(tt + 1) * 128],
                                     2 * t + 1, 2))

                    for idx, (lhsT, rhs, c_lo, c_w) in enumerate(jobs):
                        nc.tensor.matmul(
                            ctx[0:65, (c_lo - l0) * BLK:(c_lo - l0 + c_w) * BLK],
                            lhsT=lhsT, rhs=rhs,
                            start=(idx == 0), stop=(idx == len(jobs) - 1),
                        )

                    og = aux.tile([128, 512], f32, name=f"og{p}_{g}", tag="outstage")
                    nc.vector.tensor_copy(og[0:65, 0:W], ctx[0:65, 0:W])
                    nc.sync.dma_start(out=d_out[p][:, l0 * BLK: l0 * BLK + W],
                                      in_=og[0:65, 0:W])

    if apply_fixup:
        _fixup_multiwait(nc, mybir)
    return nc


def _get_program():
    if "nc" not in _COMPILED:
        _COMPILED["nc"] = _build_program()
    return _COMPILED["nc"]


def kernel(query_layer, key_layer, value_layer, band_mask, from_mask, to_mask,
           from_blocked_mask, to_blocked_mask, rand_attn):
    import sys
    if "/opt/trn_rl_repo" not in sys.path:
        sys.path.insert(0, "/opt/trn_rl_repo")
    from concourse.bass_utils import run_bass_kernel_spmd

    arrs = _build_host_arrays(query_layer, key_layer, value_layer, rand_attn)
    nc = _get_program()

    in_maps = []
    for c in range(NCORE):
        sl = slice(c * PPC, (c + 1) * PPC)
        in_maps.append({k: np.ascontiguousarray(v[sl]) for k, v in arrs.items()})

    res = run_bass_kernel_spmd(nc, in_maps, list(range(NCORE)))

    outs = np.stack([res.results[c]["out"] for c in range(NCORE)])  # [8,3,65,S]
    outs = outs.reshape(NPAIR, 65, S).astype(np.float64)
    ctx = outs[:, :64, :] / outs[:, 64:65, :]                        # [24, 64, S]
    ctx = ctx.transpose(0, 2, 1).reshape(B, H, S, D)                 # [B,H,S,D]
    out = ctx.transpose(0, 2, 1, 3).astype(np.float32)               # [B,S,H,D]
    return np.ascontiguousarray(out)



# revision 5
# speedup vs baseline: 1.0331x; 1.0331x over previous
"""BigBird-style block-sparse attention on 8 Trainium2 NeuronCores.

Problem: B=2, H=12, S=4096, D=64, BLK=64 (64 blocks), R=3 random blocks.
All mask inputs are ones (per the generator spec); rand_attn drives the
gather structure and is read host-side.

Sharding: 24 (b,h) pairs -> 3 per core (data + head parallel).

Per-pair algorithm, all in "ST" layout (keys on PSUM partitions, queries on
the free axis):
  - middle blocks l=1..62 attend exactly 8 key blocks, organized as 4 packs
    of 128 keys: A=(l-1,l) [a kt slice], B=(l+1,r0), C=(r1,r2) [host
    gathered], G=(0,63) [shared; l=1/l=62 edge duplicates removed by
    zeroed V variants vg1/vg62, so no device-side masking at all].
  - l=0,63 attend densely to all keys (32 shared v packs).
  QK matmuls produce scores in PSUM, ACT does exp (scale and -2 bias fused;
  the bias cancels in the softmax ratio), PV matmuls contract keys with a
  ones-column appended to V so the denominator accumulates in out row 64.
  Output is the unnormalized ctx^T [65, 4096] bf16 per pair; the host
  divides by row 64 and transposes.

Emission is software-pipelined (QK of group g+1 issues before PV of group
g) so the PE never waits on the ACT engine and the DVFS clock stays high.
"""

import numpy as np

B, H, S, D = 2, 12, 4096, 64
BLK = 64
NB = S // BLK            # 64
R = 3
NPAIR = B * H            # 24
NCORE = 8
PPC = NPAIR // NCORE     # 3 pairs per core
NMID = 62                # l = 1..62
SCALE = 0.125            # 1/sqrt(64)
EBIAS = -2.0             # exp(s*SCALE + EBIAS): cancels in softmax ratio

# middle groups: 10 groups of 6 + 1 group of 2  (l = 1..62)
GROUPS = [(1 + 6 * g, 6) for g in range(10)] + [(61, 2)]

_COMPILED = {}


def _build_host_arrays(query_layer, key_layer, value_layer, rand_attn):
    import ml_dtypes
    bf16 = ml_dtypes.bfloat16

    q = np.ascontiguousarray(query_layer, dtype=np.float32).reshape(NPAIR, S, D)
    k = np.ascontiguousarray(key_layer, dtype=np.float32).reshape(NPAIR, S, D)
    v = np.ascontiguousarray(value_layer, dtype=np.float32).reshape(NPAIR, S, D)
    r = np.ascontiguousarray(rand_attn, dtype=np.int64).reshape(NPAIR, NMID, R)

    qt = np.ascontiguousarray(q.transpose(0, 2, 1)).astype(bf16)   # [24,64,S]
    kt = np.ascontiguousarray(k.transpose(0, 2, 1)).astype(bf16)   # [24,64,S]

    # dense q blocks {0, 63}: [24, 64, 128]
    qtd = np.concatenate([qt[:, :, 0:BLK], qt[:, :, S - BLK:]], axis=2)
    qtd = np.ascontiguousarray(qtd)
    # global key pack {0, 63}: [24, 64, 128]
    ktg = np.concatenate([kt[:, :, 0:BLK], kt[:, :, S - BLK:]], axis=2)
    ktg = np.ascontiguousarray(ktg)

    # kr: per-l gathered packs B=(l+1, r0), C=(r1, r2): [24, 64, 62*256]
    kb = kt.reshape(NPAIR, D, NB, BLK)                # [24, 64, 64, 64]
    bh = np.arange(NPAIR)[:, None, None]
    ls = np.arange(1, NMID + 1)                       # l = 1..62
    blocks = np.empty((NPAIR, NMID, 4), np.int64)
    blocks[:, :, 0] = ls[None, :] + 1                 # l+1
    blocks[:, :, 1:] = r                              # r0, r1, r2
    kr = kb[bh, :, blocks]                            # -> [24, 62, 4, 64, 64]? check
    # fancy index: kb[bh(24,1,1), :, blocks(24,62,4)] -> [24, 62, 4, 64, 64]
    kr = np.ascontiguousarray(kr.transpose(0, 3, 1, 2, 4)
                              .reshape(NPAIR, D, NMID * 4 * BLK))

    ones = np.ones((NPAIR, NB, BLK, 1), np.float32)
    v65 = np.concatenate([v.reshape(NPAIR, NB, BLK, D), ones], axis=3)  # [24,64,64,65]

    # vw: all consecutive-pair v packs j=0..62: keys j*64 .. j*64+128
    # [24, 63, 128, 65] -> [24, 128, 63*65]
    v65f = v65.reshape(NPAIR, NB * BLK, D + 1)
    idx = (np.arange(63)[:, None] * BLK + np.arange(128)[None, :])      # [63,128]
    vw = v65f[:, idx]                                 # [24, 63, 128, 65]
    vw = np.ascontiguousarray(vw.transpose(0, 2, 1, 3)
                              .reshape(NPAIR, 128, 63 * (D + 1))).astype(bf16)

    # vr: per-l packs B=(v_{l+1}, v_{r0}), C=(v_{r1}, v_{r2}):
    # [24, 62, 4, 64, 65] -> pairs -> [24, 128, 62*2*65]
    vr = v65[bh, blocks]                              # [24, 62, 4, 64, 65]
    vr = vr.reshape(NPAIR, NMID, 2, 2, BLK, D + 1)    # [24,62,2pack,2half,64,65]
    vr = vr.reshape(NPAIR, NMID, 2, 128, D + 1)
    vr = np.ascontiguousarray(vr.transpose(0, 3, 1, 2, 4)
                              .reshape(NPAIR, 128, NMID * 2 * (D + 1))).astype(bf16)

    # global v pack {0, 63} + edge variants
    vg_full = np.concatenate([v65[:, 0], v65[:, NB - 1]], axis=1)  # [24,128,65]
    vg1 = vg_full.copy()
    vg1[:, 0:BLK, :] = 0.0        # l=1: block 0 already in its window pack A
    vg62 = vg_full.copy()
    vg62[:, BLK:, :] = 0.0        # l=62: block 63 already in its pack B
    vg = np.ascontiguousarray(vg_full).astype(bf16)
    vg1 = np.ascontiguousarray(vg1).astype(bf16)
    vg62 = np.ascontiguousarray(vg62).astype(bf16)

    return dict(qt=qt, kt=kt, qtd=qtd.astype(bf16), ktg=ktg.astype(bf16),
                kr=kr.astype(bf16), vw=vw, vr=vr, vg=vg, vg1=vg1, vg62=vg62)


def _fixup_multiwait(nc, mybir):
    """Split >1-sem-wait instructions (the Tile exit drain) into single-wait
    NoOps: this walrus build's CTRL codegen has one wait slot."""
    for fn in nc.m.functions:
        for bb in fn.blocks:
            insts = list(bb.instructions)
            out = []
            for inst in insts:
                si = inst.sync_info
                if si is not None and len(si.on_wait) > 1:
                    waits = list(si.on_wait)
                    for kk, w in enumerate(waits[:-1]):
                        nop = mybir.InstNoOp(
                            name=f"{inst.name}-wsplit{kk}",
                            opcode="NoOp",
                            engine=inst.engine,
                            sync_info=mybir.SyncInfo(on_wait=[w], on_update=[]),
                        )
                        out.append(nop)
                    si.on_wait = [waits[-1]]
                    inst.sync_info = si
                out.append(inst)
            bb.instructions = out


def _build_program(apply_fixup=True):
    import sys
    if "/opt/trn_rl_repo" not in sys.path:
        sys.path.insert(0, "/opt/trn_rl_repo")
    import concourse.bass as bass
    import concourse.mybir as mybir
    from concourse.tile import TileContext

    f32 = mybir.dt.float32
    bf16 = mybir.dt.bfloat16
    EXP = mybir.ActivationFunctionType.Exp

    nc = bass.Bass("TRN2", target_bir_lowering=False, debug=False,
                   num_devices=NCORE)

    # register a const AP for the exp bias
    _bias_t = nc.alloc_sbuf_tensor("const-f32-ebias", [128, 1], f32)
    nc.gpsimd.memset(_bias_t.ap(), EBIAS)
    nc.const_aps.aps[(f32, EBIAS)] = _bias_t.ap()
    nc.all_engine_barrier()

    d_qt = nc.dram_tensor("qt", [PPC, D, S], bf16, kind="ExternalInput").ap()
    d_kt = nc.dram_tensor("kt", [PPC, D, S], bf16, kind="ExternalInput").ap()
    d_qtd = nc.dram_tensor("qtd", [PPC, D, 128], bf16, kind="ExternalInput").ap()
    d_ktg = nc.dram_tensor("ktg", [PPC, D, 128], bf16, kind="ExternalInput").ap()
    d_kr = nc.dram_tensor("kr", [PPC, D, NMID * 256], bf16, kind="ExternalInput").ap()
    d_vw = nc.dram_tensor("vw", [PPC, 128, 63 * 65], bf16, kind="ExternalInput").ap()
    d_vr = nc.dram_tensor("vr", [PPC, 128, NMID * 2 * 65], bf16, kind="ExternalInput").ap()
    d_vg = nc.dram_tensor("vg", [PPC, 128, 65], bf16, kind="ExternalInput").ap()
    d_vg1 = nc.dram_tensor("vg1", [PPC, 128, 65], bf16, kind="ExternalInput").ap()
    d_vg62 = nc.dram_tensor("vg62", [PPC, 128, 65], bf16, kind="ExternalInput").ap()
    d_out = nc.dram_tensor("out", [PPC, 65, S], bf16, kind="ExternalOutput").ap()

    with TileContext(nc) as tc:
        with tc.tile_pool(name="sb", bufs=2) as sb, \
             tc.tile_pool(name="ps", bufs=2, space="PSUM") as ps, \
             tc.tile_pool(name="ptp", bufs=4) as ptp, \
             tc.tile_pool(name="aux", bufs=4) as aux:

            for p in range(PPC):
                qt = sb.tile([D, S], bf16, name=f"qt{p}", tag="qt")
                kt = sb.tile([D, S], bf16, name=f"kt{p}", tag="kt")
                qtd = sb.tile([D, 128], bf16, name=f"qtd{p}", tag="qtd")
                ktg = sb.tile([D, 128], bf16, name=f"ktg{p}", tag="ktg")
                kr = sb.tile([D, NMID * 256], bf16, name=f"kr{p}", tag="kr")
                vw = sb.tile([128, 63 * 65], bf16, name=f"vw{p}", tag="vw")
                vr = sb.tile([128, NMID * 2 * 65], bf16, name=f"vr{p}", tag="vr")
                vg = sb.tile([128, 65], bf16, name=f"vg{p}", tag="vg")
                vg1 = sb.tile([128, 65], bf16, name=f"vg1{p}", tag="vg1")
                vg62 = sb.tile([128, 65], bf16, name=f"vg62{p}", tag="vg62")

                # input DMAs spread over engine queues; small/first-needed on
                # sync, bulk split elsewhere
                for t_, d_ in ((qtd, d_qtd), (ktg, d_ktg), (vg, d_vg),
                               (vg1, d_vg1), (vg62, d_vg62)):
                    nc.sync.dma_start(out=t_, in_=d_[p])
                nc.scalar.dma_start(out=kt, in_=d_kt[p])
                nc.sync.dma_start(out=vw, in_=d_vw[p])
                nc.scalar.dma_start(out=qt, in_=d_qt[p])
                h_kr = NMID * 128
                nc.sync.dma_start(out=kr[:, 0:h_kr], in_=d_kr[p][:, 0:h_kr])
                nc.gpsimd.dma_start(out=kr[:, h_kr:], in_=d_kr[p][:, h_kr:])
                h_vr = NMID * 65
                nc.gpsimd.dma_start(out=vr[:, 0:h_vr], in_=d_vr[p][:, 0:h_vr])
                nc.gpsimd.dma_start(out=vr[:, h_vr:], in_=d_vr[p][:, h_vr:])

                def vw_pack(j):                     # keys j*64 .. j*64+128
                    return vw[:, j * 65:(j + 1) * 65]

                # ---- stage emitters (software pipeline) ----
                stages = []

                # dense stage c: 8 key packs (even j = 16c..16c+14)
                def dense_qk(c):
                    st = ps.tile([128, 1536], f32, name=f"std{p}_{c}", tag="st")
                    for j in range(8):
                        nc.tensor.matmul(
                            st[:, j * 128:(j + 1) * 128],
                            lhsT=kt[:, (8 * c + j) * 128:(8 * c + j + 1) * 128],
                            rhs=qtd, start=True, stop=True)
                    pt = ptp.tile([128, 1536], bf16, name=f"ptd{p}_{c}", tag="pt")
                    nc.scalar.activation(pt[:, 0:1024], st[:, 0:1024], EXP,
                                         scale=SCALE, bias=EBIAS)
                    return st, pt

                def dense_pv(c, st_pt, ctxd):
                    st, pt = st_pt
                    for j in range(8):
                        nc.tensor.matmul(
                            ctxd[0:65, 0:128],
                            lhsT=vw_pack(2 * (8 * c + j)),
                            rhs=pt[:, j * 128:(j + 1) * 128],
                            start=(c == 0 and j == 0), stop=(c == 3 and j == 7))

                # middle group g: nl blocks starting at l0
                def group_qk(g):
                    l0, nl = GROUPS[g]
                    st = ps.tile([128, 1536], f32, name=f"st{p}_{g}", tag="st")
                    for j in range(nl):
                        l = l0 + j
                        o = j * 256
                        rhs = qt[:, l * BLK:(l + 1) * BLK]
                        # A = (l-1, l) straight from kt
                        nc.tensor.matmul(st[:, o:o + 64],
                                         lhsT=kt[:, (l - 1) * BLK:(l + 1) * BLK],
                                         rhs=rhs, start=True, stop=True)
                        # B = (l+1, r0), C = (r1, r2)
                        kb_ = kr[:, (l - 1) * 256:(l - 1) * 256 + 128]
                        kc_ = kr[:, (l - 1) * 256 + 128:(l - 1) * 256 + 256]
                        nc.tensor.matmul(st[:, o + 64:o + 128], lhsT=kb_,
                                         rhs=rhs, start=True, stop=True)
                        nc.tensor.matmul(st[:, o + 128:o + 192], lhsT=kc_,
                                         rhs=rhs, start=True, stop=True)
                        # G = (0, 63)
                        nc.tensor.matmul(st[:, o + 192:o + 256], lhsT=ktg,
                                         rhs=rhs, start=True, stop=True)
                    pt = ptp.tile([128, 1536], bf16, name=f"pt{p}_{g}", tag="pt")
                    nc.scalar.activation(pt[:, 0:nl * 256], st[:, 0:nl * 256],
                                         EXP, scale=SCALE, bias=EBIAS)
                    return st, pt

                def group_pv(g, st_pt, ctx):
                    st, pt = st_pt
                    l0, nl = GROUPS[g]
                    for j in range(nl):
                        l = l0 + j
                        o = j * 256
                        oc = j * BLK
                        vb_ = vr[:, (l - 1) * 130:(l - 1) * 130 + 65]
                        vc_ = vr[:, (l - 1) * 130 + 65:(l - 1) * 130 + 130]
                        vg_ = vg1 if l == 1 else (vg62 if l == 62 else vg)
                        nc.tensor.matmul(ctx[0:65, oc:oc + 64],
                                         lhsT=vw_pack(l - 1),
                                         rhs=pt[:, o:o + 64],
                                         start=True, stop=False)
                        nc.tensor.matmul(ctx[0:65, oc:oc + 64], lhsT=vb_,
                                         rhs=pt[:, o + 64:o + 128],
                                         start=False, stop=False)
                        nc.tensor.matmul(ctx[0:65, oc:oc + 64], lhsT=vc_,
                                         rhs=pt[:, o + 128:o + 192],
                                         start=False, stop=False)
                        nc.tensor.matmul(ctx[0:65, oc:oc + 64], lhsT=vg_,
                                         rhs=pt[:, o + 192:o + 256],
                                         start=False, stop=True)

                def group_out(g, ctx):
                    l0, nl = GROUPS[g]
                    w = nl * BLK
                    og = aux.tile([128, 384], bf16, name=f"og{p}_{g}", tag="og")
                    nc.vector.tensor_copy(og[0:65, 0:w], ctx[0:65, 0:w])
                    nc.sync.dma_start(out=d_out[p][:, l0 * BLK:l0 * BLK + w],
                                      in_=og[0:65, 0:w])

                # ---- emit: dense chunks + groups, depth-1 pipelined ----
                ctxd = ps.tile([128, 128], f32, name=f"ctxd{p}", tag="ctx")
                prev = None          # (kind, idx, st_pt)
                ctx_of = {}
                for stage in ([("d", c) for c in range(4)] +
                              [("g", g) for g in range(len(GROUPS))]):
                    kind, i = stage
                    cur = (kind, i, dense_qk(i) if kind == "d" else group_qk(i))
                    if prev is not None:
                        pk, pi, pst = prev
                        if pk == "d":
                            dense_pv(pi, pst, ctxd)
                            if pi == 3:
                                od = aux.tile([128, 384], bf16,
                                              name=f"od{p}", tag="og")
                                nc.vector.tensor_copy(od[0:65, 0:128],
                                                      ctxd[0:65, 0:128])
                                nc.sync.dma_start(out=d_out[p][:, 0:BLK],
                                                  in_=od[0:65, 0:64])
                                nc.sync.dma_start(out=d_out[p][:, S - BLK:],
                                                  in_=od[0:65, 64:128])
                        else:
                            group_pv(pi, pst, ctx_of[pi])
                            group_out(pi, ctx_of[pi])
                    if kind == "g":
                        l0, nl = GROUPS[i]
                        ctx_of[i] = ps.tile([128, 384], f32,
                                            name=f"ctx{p}_{i}", tag="ctx")
                    prev = cur
                # drain last stage
                pk, pi, pst = prev
                group_pv(pi, pst, ctx_of[pi])
                group_out(pi, ctx_of[pi])

    if apply_fixup:
        _fixup_multiwait(nc, mybir)
    return nc


def _get_program():
    if "nc" not in _COMPILED:
        _COMPILED["nc"] = _build_program()
    return _COMPILED["nc"]


def kernel(query_layer, key_layer, value_layer, band_mask, from_mask, to_mask,
           from_blocked_mask, to_blocked_mask, rand_attn):
    import sys
    if "/opt/trn_rl_repo" not in sys.path:
        sys.path.insert(0, "/opt/trn_rl_repo")
    from concourse.bass_utils import run_bass_kernel_spmd

    arrs = _build_host_arrays(query_layer, key_layer, value_layer, rand_attn)
    nc = _get_program()

    in_maps = []
    for c in range(NCORE):
        sl = slice(c * PPC, (c + 1) * PPC)
        in_maps.append({k: np.ascontiguousarray(v[sl]) for k, v in arrs.items()})

    res = run_bass_kernel_spmd(nc, in_maps, list(range(NCORE)))

    outs = np.stack([np.asarray(res.results[c]["out"]) for c in range(NCORE)])
    outs = outs.reshape(NPAIR, 65, S).astype(np.float64)
    ctx = outs[:, :64, :] / outs[:, 64:65, :]                        # [24,64,S]
    ctx = ctx.transpose(0, 2, 1).reshape(B, H, S, D)                 # [B,H,S,D]
    out = ctx.transpose(0, 2, 1, 3).astype(np.float32)               # [B,S,H,D]
    return np.ascontiguousarray(out)


# revision 7
# speedup vs baseline: 1.1262x; 1.0901x over previous
"""BigBird-style block-sparse attention on 8 Trainium2 NeuronCores.

Problem: B=2, H=12, S=4096, D=64, BLK=64 (64 blocks), R=3 random blocks.
All mask inputs are ones (per the generator spec); rand_attn drives the
gather structure and is read host-side.

Sharding: 24 (b,h) pairs -> 3 per core (data + head parallel).

Per-pair algorithm, all in "ST" layout (keys on PSUM partitions, queries on
the free axis):
  - middle blocks l=1..62 attend exactly 8 key blocks, organized as 4 packs
    of 128 keys: A=(l-1,l) [a kt slice], B=(l+1,r0), C=(r1,r2) [host
    gathered], G=(0,63) [shared; l=1/l=62 edge duplicates removed by
    zeroed V variants vg1/vg62, so no device-side masking at all].
  - l=0,63 attend densely to all keys (32 shared v packs).
  QK matmuls produce scores in PSUM, ACT does exp (scale and -2 bias fused;
  the bias cancels in the softmax ratio), PV matmuls contract keys with a
  ones-column appended to V so the denominator accumulates in out row 64.
  Output is the unnormalized ctx^T [65, 4096] bf16 per pair; the host
  divides by row 64 and transposes.

Emission is software-pipelined (QK of group g+1 issues before PV of group
g) so the PE never waits on the ACT engine and the DVFS clock stays high.
"""

import numpy as np

B, H, S, D = 2, 12, 4096, 64
BLK = 64
NB = S // BLK            # 64
R = 3
NPAIR = B * H            # 24
NCORE = 8
PPC = NPAIR // NCORE     # 3 pairs per core
NMID = 62                # l = 1..62
SCALE = 0.125            # 1/sqrt(64)
EBIAS = -2.0             # exp(s*SCALE + EBIAS): cancels in softmax ratio

# middle groups: 15 groups of 4 + 1 group of 2  (l = 1..62)
GROUPS = [(1 + 4 * g, 4) for g in range(15)] + [(61, 2)]

_COMPILED = {}


def _build_host_arrays(query_layer, key_layer, value_layer, rand_attn):
    import ml_dtypes
    bf16 = ml_dtypes.bfloat16

    q = np.ascontiguousarray(query_layer, dtype=np.float32).reshape(NPAIR, S, D)
    k = np.ascontiguousarray(key_layer, dtype=np.float32).reshape(NPAIR, S, D)
    v = np.ascontiguousarray(value_layer, dtype=np.float32).reshape(NPAIR, S, D)
    r = np.ascontiguousarray(rand_attn, dtype=np.int64).reshape(NPAIR, NMID, R)

    qt = np.ascontiguousarray(q.transpose(0, 2, 1)).astype(bf16)   # [24,64,S]
    kt = np.ascontiguousarray(k.transpose(0, 2, 1)).astype(bf16)   # [24,64,S]

    # dense q blocks {0, 63}: [24, 64, 128]
    qtd = np.concatenate([qt[:, :, 0:BLK], qt[:, :, S - BLK:]], axis=2)
    qtd = np.ascontiguousarray(qtd)
    # global key pack {0, 63}: [24, 64, 128]
    ktg = np.concatenate([kt[:, :, 0:BLK], kt[:, :, S - BLK:]], axis=2)
    ktg = np.ascontiguousarray(ktg)

    # kr: per-l gathered packs B=(l+1, r0), C=(r1, r2): [24, 64, 62*256]
    kb = kt.reshape(NPAIR, D, NB, BLK)                # [24, 64, 64, 64]
    bh = np.arange(NPAIR)[:, None, None]
    ls = np.arange(1, NMID + 1)                       # l = 1..62
    blocks = np.empty((NPAIR, NMID, 4), np.int64)
    blocks[:, :, 0] = ls[None, :] + 1                 # l+1
    blocks[:, :, 1:] = r                              # r0, r1, r2
    kr = kb[bh, :, blocks]                            # -> [24, 62, 4, 64, 64]? check
    # fancy index: kb[bh(24,1,1), :, blocks(24,62,4)] -> [24, 62, 4, 64, 64]
    kr = np.ascontiguousarray(kr.transpose(0, 3, 1, 2, 4)
                              .reshape(NPAIR, D, NMID * 4 * BLK))

    ones = np.ones((NPAIR, NB, BLK, 1), np.float32)
    v65 = np.concatenate([v.reshape(NPAIR, NB, BLK, D), ones], axis=3)  # [24,64,64,65]

    # vw: all consecutive-pair v packs j=0..62: keys j*64 .. j*64+128
    # [24, 63, 128, 65] -> [24, 128, 63*65]
    v65f = v65.reshape(NPAIR, NB * BLK, D + 1)
    idx = (np.arange(63)[:, None] * BLK + np.arange(128)[None, :])      # [63,128]
    vw = v65f[:, idx]                                 # [24, 63, 128, 65]
    vw = np.ascontiguousarray(vw.transpose(0, 2, 1, 3)
                              .reshape(NPAIR, 128, 63 * (D + 1))).astype(bf16)

    # vr: per-l packs B=(v_{l+1}, v_{r0}), C=(v_{r1}, v_{r2}):
    # [24, 62, 4, 64, 65] -> pairs -> [24, 128, 62*2*65]
    vr = v65[bh, blocks]                              # [24, 62, 4, 64, 65]
    vr = vr.reshape(NPAIR, NMID, 2, 2, BLK, D + 1)    # [24,62,2pack,2half,64,65]
    vr = vr.reshape(NPAIR, NMID, 2, 128, D + 1)
    vr = np.ascontiguousarray(vr.transpose(0, 3, 1, 2, 4)
                              .reshape(NPAIR, 128, NMID * 2 * (D + 1))).astype(bf16)

    # global v pack {0, 63} + edge variants
    vg_full = np.concatenate([v65[:, 0], v65[:, NB - 1]], axis=1)  # [24,128,65]
    vg1 = vg_full.copy()
    vg1[:, 0:BLK, :] = 0.0        # l=1: block 0 already in its window pack A
    vg62 = vg_full.copy()
    vg62[:, BLK:, :] = 0.0        # l=62: block 63 already in its pack B
    vg = np.ascontiguousarray(vg_full).astype(bf16)
    vg1 = np.ascontiguousarray(vg1).astype(bf16)
    vg62 = np.ascontiguousarray(vg62).astype(bf16)

    return dict(qt=qt, kt=kt, qtd=qtd.astype(bf16), ktg=ktg.astype(bf16),
                kr=kr.astype(bf16), vw=vw, vr=vr, vg=vg, vg1=vg1, vg62=vg62)


def _fixup_multiwait(nc, mybir):
    """Split >1-sem-wait instructions (the Tile exit drain) into single-wait
    NoOps: this walrus build's CTRL codegen has one wait slot."""
    for fn in nc.m.functions:
        for bb in fn.blocks:
            insts = list(bb.instructions)
            out = []
            for inst in insts:
                si = inst.sync_info
                if si is not None and len(si.on_wait) > 1:
                    waits = list(si.on_wait)
                    for kk, w in enumerate(waits[:-1]):
                        nop = mybir.InstNoOp(
                            name=f"{inst.name}-wsplit{kk}",
                            opcode="NoOp",
                            engine=inst.engine,
                            sync_info=mybir.SyncInfo(on_wait=[w], on_update=[]),
                        )
                        out.append(nop)
                    si.on_wait = [waits[-1]]
                    inst.sync_info = si
                out.append(inst)
            bb.instructions = out


def _build_program(apply_fixup=True):
    import sys
    if "/opt/trn_rl_repo" not in sys.path:
        sys.path.insert(0, "/opt/trn_rl_repo")
    import concourse.bass as bass
    import concourse.mybir as mybir
    from concourse.tile import TileContext

    f32 = mybir.dt.float32
    bf16 = mybir.dt.bfloat16
    EXP = mybir.ActivationFunctionType.Exp

    nc = bass.Bass("TRN2", target_bir_lowering=False, debug=False,
                   num_devices=NCORE)

    # register a const AP for the exp bias
    _bias_t = nc.alloc_sbuf_tensor("const-f32-ebias", [128, 1], f32)
    nc.gpsimd.memset(_bias_t.ap(), EBIAS)
    nc.const_aps.aps[(f32, EBIAS)] = _bias_t.ap()
    nc.all_engine_barrier()

    d_qt = nc.dram_tensor("qt", [PPC, D, S], bf16, kind="ExternalInput").ap()
    d_kt = nc.dram_tensor("kt", [PPC, D, S], bf16, kind="ExternalInput").ap()
    d_qtd = nc.dram_tensor("qtd", [PPC, D, 128], bf16, kind="ExternalInput").ap()
    d_ktg = nc.dram_tensor("ktg", [PPC, D, 128], bf16, kind="ExternalInput").ap()
    d_kr = nc.dram_tensor("kr", [PPC, D, NMID * 256], bf16, kind="ExternalInput").ap()
    d_vw = nc.dram_tensor("vw", [PPC, 128, 63 * 65], bf16, kind="ExternalInput").ap()
    d_vr = nc.dram_tensor("vr", [PPC, 128, NMID * 2 * 65], bf16, kind="ExternalInput").ap()
    d_vg = nc.dram_tensor("vg", [PPC, 128, 65], bf16, kind="ExternalInput").ap()
    d_vg1 = nc.dram_tensor("vg1", [PPC, 128, 65], bf16, kind="ExternalInput").ap()
    d_vg62 = nc.dram_tensor("vg62", [PPC, 128, 65], bf16, kind="ExternalInput").ap()
    d_out = nc.dram_tensor("out", [PPC, 65, S], bf16, kind="ExternalOutput").ap()

    with TileContext(nc) as tc:
        with tc.tile_pool(name="sb", bufs=2) as sb, \
             tc.tile_pool(name="ps", bufs=2, space="PSUM") as ps, \
             tc.tile_pool(name="ptp", bufs=4) as ptp, \
             tc.tile_pool(name="aux", bufs=2) as aux:

            for p in range(PPC):
                # K-side and Q-side tiles are 128 partitions tall: rows 0:64
                # carry data (DMA), rows 64:128 are zeroed so every matmul
                # contracts K=128 (the tensor engine only clocks up under
                # full-partition contractions).
                qt = sb.tile([128, S], bf16, name=f"qt{p}", tag="qt")
                kt = sb.tile([128, S], bf16, name=f"kt{p}", tag="kt")
                qtd = sb.tile([128, 128], bf16, name=f"qtd{p}", tag="qtd")
                ktg = sb.tile([128, 128], bf16, name=f"ktg{p}", tag="ktg")
                kr = sb.tile([128, NMID * 256], bf16, name=f"kr{p}", tag="kr")
                vw = sb.tile([128, 63 * 65], bf16, name=f"vw{p}", tag="vw")
                vr = sb.tile([128, NMID * 2 * 65], bf16, name=f"vr{p}", tag="vr")
                vg = sb.tile([128, 65], bf16, name=f"vg{p}", tag="vg")
                vg1 = sb.tile([128, 65], bf16, name=f"vg1{p}", tag="vg1")
                vg62 = sb.tile([128, 65], bf16, name=f"vg62{p}", tag="vg62")

                for t_, d_ in ((qtd, d_qtd), (ktg, d_ktg)):
                    nc.sync.dma_start(out=t_[0:64, :], in_=d_[p])
                for t_, d_ in ((vg, d_vg), (vg1, d_vg1), (vg62, d_vg62)):
                    nc.sync.dma_start(out=t_, in_=d_[p])
                nc.sync.dma_start(out=kt[0:64, :], in_=d_kt[p])
                nc.sync.dma_start(out=vw, in_=d_vw[p])
                nc.sync.dma_start(out=qt[0:64, :], in_=d_qt[p])
                h_kr = NMID * 128
                nc.gpsimd.dma_start(out=kr[0:64, 0:h_kr], in_=d_kr[p][:, 0:h_kr])
                nc.gpsimd.dma_start(out=kr[0:64, h_kr:], in_=d_kr[p][:, h_kr:])
                h_vr = NMID * 65
                nc.gpsimd.dma_start(out=vr[:, 0:h_vr], in_=d_vr[p][:, 0:h_vr])
                nc.gpsimd.dma_start(out=vr[:, h_vr:], in_=d_vr[p][:, h_vr:])

                # zero the stacked contraction rows (cheap on DVE, overlapped)
                nc.vector.memset(qtd[64:128, :], 0.0)
                nc.vector.memset(ktg[64:128, :], 0.0)
                nc.vector.memset(kt[64:128, :], 0.0)
                nc.vector.memset(qt[64:128, :], 0.0)
                nc.vector.memset(kr[64:128, :], 0.0)

                ostage = aux.tile([65, S], bf16, name=f"ostage{p}", tag="os")

                def vw_pack(j):                     # keys j*64 .. j*64+128
                    return vw[:, j * 65:(j + 1) * 65]

                # dense stage c: 8 key packs (even j = 16c..16c+14)
                def dense_qk(c):
                    st = ps.tile([128, 1024], f32, name=f"std{p}_{c}", tag="st",
                                 bufs=3)
                    for j in range(8):
                        nc.tensor.matmul(
                            st[:, j * 128:(j + 1) * 128],
                            lhsT=kt[:, (8 * c + j) * 128:(8 * c + j + 1) * 128],
                            rhs=qtd, start=True, stop=True)
                    pt = ptp.tile([128, 1024], bf16, name=f"ptd{p}_{c}", tag="pt")
                    nc.scalar.activation(pt, st, EXP, scale=SCALE, bias=EBIAS)
                    return st, pt

                def dense_pv(c, st_pt, ctxd):
                    st, pt = st_pt
                    for j in range(8):
                        nc.tensor.matmul(
                            ctxd[0:65, 0:128],
                            lhsT=vw_pack(2 * (8 * c + j)),
                            rhs=pt[:, j * 128:(j + 1) * 128],
                            start=(c == 0 and j == 0), stop=(c == 3 and j == 7))

                # middle group g: nl blocks starting at l0
                def group_qk(g):
                    l0, nl = GROUPS[g]
                    st = ps.tile([128, 1024], f32, name=f"st{p}_{g}", tag="st",
                                 bufs=3)
                    for j in range(nl):
                        l = l0 + j
                        o = j * 256
                        rhs = qt[:, l * BLK:(l + 1) * BLK]
                        nc.tensor.matmul(st[:, o:o + 64],
                                         lhsT=kt[:, (l - 1) * BLK:(l + 1) * BLK],
                                         rhs=rhs, start=True, stop=True)
                        kb_ = kr[:, (l - 1) * 256:(l - 1) * 256 + 128]
                        kc_ = kr[:, (l - 1) * 256 + 128:(l - 1) * 256 + 256]
                        nc.tensor.matmul(st[:, o + 64:o + 128], lhsT=kb_,
                                         rhs=rhs, start=True, stop=True)
                        nc.tensor.matmul(st[:, o + 128:o + 192], lhsT=kc_,
                                         rhs=rhs, start=True, stop=True)
                        nc.tensor.matmul(st[:, o + 192:o + 256], lhsT=ktg,
                                         rhs=rhs, start=True, stop=True)
                    pt = ptp.tile([128, 1024], bf16, name=f"pt{p}_{g}", tag="pt")
                    nc.scalar.activation(pt[:, 0:nl * 256], st[:, 0:nl * 256],
                                         EXP, scale=SCALE, bias=EBIAS)
                    return st, pt

                def group_pv(g, st_pt, ctx):
                    st, pt = st_pt
                    l0, nl = GROUPS[g]
                    for j in range(nl):
                        l = l0 + j
                        o = j * 256
                        oc = j * BLK
                        vb_ = vr[:, (l - 1) * 130:(l - 1) * 130 + 65]
                        vc_ = vr[:, (l - 1) * 130 + 65:(l - 1) * 130 + 130]
                        vg_ = vg1 if l == 1 else (vg62 if l == 62 else vg)
                        nc.tensor.matmul(ctx[0:65, oc:oc + 64],
                                         lhsT=vw_pack(l - 1),
                                         rhs=pt[:, o:o + 64],
                                         start=True, stop=False)
                        nc.tensor.matmul(ctx[0:65, oc:oc + 64], lhsT=vb_,
                                         rhs=pt[:, o + 64:o + 128],
                                         start=False, stop=False)
                        nc.tensor.matmul(ctx[0:65, oc:oc + 64], lhsT=vc_,
                                         rhs=pt[:, o + 128:o + 192],
                                         start=False, stop=False)
                        nc.tensor.matmul(ctx[0:65, oc:oc + 64], lhsT=vg_,
                                         rhs=pt[:, o + 192:o + 256],
                                         start=False, stop=True)

                def group_out(g, ctx):
                    l0, nl = GROUPS[g]
                    w = nl * BLK
                    nc.vector.tensor_copy(ostage[:, l0 * BLK:l0 * BLK + w],
                                          ctx[0:65, 0:w])

                # ---- emit: dense chunks + groups, depth-2 pipelined ----
                ctxd = ps.tile([128, 128], f32, name=f"ctxd{p}", tag="ctx")
                stages = [("d", c) for c in range(4)] + \
                         [("g", g) for g in range(len(GROUPS))]

                def run_qk(s):
                    kind, i = s
                    if kind == "g":
                        ctx_of[i] = ps.tile([128, 256], f32,
                                            name=f"ctx{p}_{i}", tag="ctx")
                    return dense_qk(i) if kind == "d" else group_qk(i)

                def run_pv(s, st_pt):
                    kind, i = s
                    if kind == "d":
                        dense_pv(i, st_pt, ctxd)
                        if i == 3:
                            nc.vector.tensor_copy(ostage[:, 0:BLK],
                                                  ctxd[0:65, 0:64])
                            nc.vector.tensor_copy(ostage[:, S - BLK:],
                                                  ctxd[0:65, 64:128])
                    else:
                        group_pv(i, st_pt, ctx_of[i])
                        group_out(i, ctx_of[i])

                ctx_of = {}
                pend = []
                for s in stages:
                    pend.append((s, run_qk(s)))
                    if len(pend) > 2:
                        ps_, st_ = pend.pop(0)
                        run_pv(ps_, st_)
                for ps_, st_ in pend:
                    run_pv(ps_, st_)

                nc.sync.dma_start(out=d_out[p], in_=ostage)

    if apply_fixup:
        _fixup_multiwait(nc, mybir)
    return nc


def _get_program():
    if "nc" not in _COMPILED:
        _COMPILED["nc"] = _build_program()
    return _COMPILED["nc"]


def kernel(query_layer, key_layer, value_layer, band_mask, from_mask, to_mask,
           from_blocked_mask, to_blocked_mask, rand_attn):
    import sys
    if "/opt/trn_rl_repo" not in sys.path:
        sys.path.insert(0, "/opt/trn_rl_repo")
    from concourse.bass_utils import run_bass_kernel_spmd

    arrs = _build_host_arrays(query_layer, key_layer, value_layer, rand_attn)
    nc = _get_program()

    in_maps = []
    for c in range(NCORE):
        sl = slice(c * PPC, (c + 1) * PPC)
        in_maps.append({k: np.ascontiguousarray(v[sl]) for k, v in arrs.items()})

    res = run_bass_kernel_spmd(nc, in_maps, list(range(NCORE)))

    outs = np.stack([np.asarray(res.results[c]["out"]) for c in range(NCORE)])
    outs = outs.reshape(NPAIR, 65, S).astype(np.float64)
    ctx = outs[:, :64, :] / outs[:, 64:65, :]                        # [24,64,S]
    ctx = ctx.transpose(0, 2, 1).reshape(B, H, S, D)                 # [B,H,S,D]
    out = ctx.transpose(0, 2, 1, 3).astype(np.float32)               # [B,S,H,D]
    return np.ascontiguousarray(out)


# revision 8
# speedup vs baseline: 1.2265x; 1.0890x over previous
"""BigBird-style block-sparse attention on 8 Trainium2 NeuronCores.

Problem: B=2, H=12, S=4096, D=64, BLK=64 (64 blocks), R=3 random blocks.
All mask inputs are ones (per the generator spec); rand_attn drives the
gather structure and is read host-side.

Sharding: 24 (b,h) pairs -> 3 per core (data + head parallel).

Per-pair algorithm, all in "ST" layout (keys on PSUM partitions, queries on
the free axis):
  - middle blocks l=1..62 attend exactly 8 key blocks, organized as 4 packs
    of 128 keys: A=(l-1,l) [a kt slice], B=(l+1,r0), C=(r1,r2) [host
    gathered], G=(0,63) [shared; l=1/l=62 edge duplicates removed by
    zeroed V variants vg1/vg62, so no device-side masking at all].
  - l=0,63 attend densely to all keys (32 shared v packs).
  QK matmuls produce scores in PSUM, ACT does exp (scale and -2 bias fused;
  the bias cancels in the softmax ratio), PV matmuls contract keys with a
  ones-column appended to V so the denominator accumulates in out row 64.
  Output is the unnormalized ctx^T [65, 4096] bf16 per pair; the host
  divides by row 64 and transposes.

Emission is software-pipelined (QK of group g+1 issues before PV of group
g) so the PE never waits on the ACT engine and the DVFS clock stays high.
"""

import numpy as np

B, H, S, D = 2, 12, 4096, 64
BLK = 64
NB = S // BLK            # 64
R = 3
NPAIR = B * H            # 24
NCORE = 8
PPC = NPAIR // NCORE     # 3 pairs per core
NMID = 62                # l = 1..62
SCALE = 0.125            # 1/sqrt(64)
EBIAS = -2.0             # exp(s*SCALE + EBIAS): cancels in softmax ratio

# middle groups: 15 groups of 4 + 1 group of 2  (l = 1..62)
GROUPS = [(1 + 4 * g, 4) for g in range(15)] + [(61, 2)]

_COMPILED = {}


def _build_host_arrays(query_layer, key_layer, value_layer, rand_attn):
    import ml_dtypes
    bf16 = ml_dtypes.bfloat16

    q = np.ascontiguousarray(query_layer, dtype=np.float32).reshape(NPAIR, S, D)
    k = np.ascontiguousarray(key_layer, dtype=np.float32).reshape(NPAIR, S, D)
    v = np.ascontiguousarray(value_layer, dtype=np.float32).reshape(NPAIR, S, D)
    r = np.ascontiguousarray(rand_attn, dtype=np.int64).reshape(NPAIR, NMID, R)

    qt = np.ascontiguousarray(q.transpose(0, 2, 1)).astype(bf16)   # [24,64,S]
    kt = np.ascontiguousarray(k.transpose(0, 2, 1)).astype(bf16)   # [24,64,S]

    # dense q blocks {0, 63}: [24, 64, 128]
    qtd = np.concatenate([qt[:, :, 0:BLK], qt[:, :, S - BLK:]], axis=2)
    qtd = np.ascontiguousarray(qtd)
    # global key pack {0, 63}: [24, 64, 128]
    ktg = np.concatenate([kt[:, :, 0:BLK], kt[:, :, S - BLK:]], axis=2)
    ktg = np.ascontiguousarray(ktg)

    # kr: per-l gathered packs B=(l+1, r0), C=(r1, r2): [24, 64, 62*256]
    kb = kt.reshape(NPAIR, D, NB, BLK)                # [24, 64, 64, 64]
    bh = np.arange(NPAIR)[:, None, None]
    ls = np.arange(1, NMID + 1)                       # l = 1..62
    blocks = np.empty((NPAIR, NMID, 4), np.int64)
    blocks[:, :, 0] = ls[None, :] + 1                 # l+1
    blocks[:, :, 1:] = r                              # r0, r1, r2
    kr = kb[bh, :, blocks]                            # -> [24, 62, 4, 64, 64]? check
    # fancy index: kb[bh(24,1,1), :, blocks(24,62,4)] -> [24, 62, 4, 64, 64]
    kr = np.ascontiguousarray(kr.transpose(0, 3, 1, 2, 4)
                              .reshape(NPAIR, D, NMID * 4 * BLK))

    ones = np.ones((NPAIR, NB, BLK, 1), np.float32)
    v65 = np.concatenate([v.reshape(NPAIR, NB, BLK, D), ones], axis=3)  # [24,64,64,65]

    # vw: all consecutive-pair v packs j=0..62: keys j*64 .. j*64+128
    # [24, 63, 128, 65] -> [24, 128, 63*65]
    v65f = v65.reshape(NPAIR, NB * BLK, D + 1)
    idx = (np.arange(63)[:, None] * BLK + np.arange(128)[None, :])      # [63,128]
    vw = v65f[:, idx]                                 # [24, 63, 128, 65]
    vw = np.ascontiguousarray(vw.transpose(0, 2, 1, 3)
                              .reshape(NPAIR, 128, 63 * (D + 1))).astype(bf16)

    # vr: per-l packs B=(v_{l+1}, v_{r0}), C=(v_{r1}, v_{r2}):
    # [24, 62, 4, 64, 65] -> pairs -> [24, 128, 62*2*65]
    vr = v65[bh, blocks]                              # [24, 62, 4, 64, 65]
    vr = vr.reshape(NPAIR, NMID, 2, 2, BLK, D + 1)    # [24,62,2pack,2half,64,65]
    vr = vr.reshape(NPAIR, NMID, 2, 128, D + 1)
    vr = np.ascontiguousarray(vr.transpose(0, 3, 1, 2, 4)
                              .reshape(NPAIR, 128, NMID * 2 * (D + 1))).astype(bf16)

    # global v pack {0, 63} + edge variants
    vg_full = np.concatenate([v65[:, 0], v65[:, NB - 1]], axis=1)  # [24,128,65]
    vg1 = vg_full.copy()
    vg1[:, 0:BLK, :] = 0.0        # l=1: block 0 already in its window pack A
    vg62 = vg_full.copy()
    vg62[:, BLK:, :] = 0.0        # l=62: block 63 already in its pack B
    vg = np.ascontiguousarray(vg_full).astype(bf16)
    vg1 = np.ascontiguousarray(vg1).astype(bf16)
    vg62 = np.ascontiguousarray(vg62).astype(bf16)

    return dict(qt=qt, kt=kt, qtd=qtd.astype(bf16), ktg=ktg.astype(bf16),
                kr=kr.astype(bf16), vw=vw, vr=vr, vg=vg, vg1=vg1, vg62=vg62)


def _fixup_multiwait(nc, mybir):
    """Split >1-sem-wait instructions (the Tile exit drain) into single-wait
    NoOps: this walrus build's CTRL codegen has one wait slot."""
    for fn in nc.m.functions:
        for bb in fn.blocks:
            insts = list(bb.instructions)
            out = []
            for inst in insts:
                si = inst.sync_info
                if si is not None and len(si.on_wait) > 1:
                    waits = list(si.on_wait)
                    for kk, w in enumerate(waits[:-1]):
                        nop = mybir.InstNoOp(
                            name=f"{inst.name}-wsplit{kk}",
                            opcode="NoOp",
                            engine=inst.engine,
                            sync_info=mybir.SyncInfo(on_wait=[w], on_update=[]),
                        )
                        out.append(nop)
                    si.on_wait = [waits[-1]]
                    inst.sync_info = si
                out.append(inst)
            bb.instructions = out


def _build_program(apply_fixup=True):
    import sys
    if "/opt/trn_rl_repo" not in sys.path:
        sys.path.insert(0, "/opt/trn_rl_repo")
    import concourse.bass as bass
    import concourse.mybir as mybir
    from concourse.tile import TileContext

    f32 = mybir.dt.float32
    bf16 = mybir.dt.bfloat16
    EXP = mybir.ActivationFunctionType.Exp

    nc = bass.Bass("TRN2", target_bir_lowering=False, debug=False,
                   num_devices=NCORE)

    # register a const AP for the exp bias
    _bias_t = nc.alloc_sbuf_tensor("const-f32-ebias", [128, 1], f32)
    nc.gpsimd.memset(_bias_t.ap(), EBIAS)
    nc.const_aps.aps[(f32, EBIAS)] = _bias_t.ap()
    nc.all_engine_barrier()

    d_qt = nc.dram_tensor("qt", [PPC, D, S], bf16, kind="ExternalInput").ap()
    d_kt = nc.dram_tensor("kt", [PPC, D, S], bf16, kind="ExternalInput").ap()
    d_qtd = nc.dram_tensor("qtd", [PPC, D, 128], bf16, kind="ExternalInput").ap()
    d_ktg = nc.dram_tensor("ktg", [PPC, D, 128], bf16, kind="ExternalInput").ap()
    d_kr = nc.dram_tensor("kr", [PPC, D, NMID * 256], bf16, kind="ExternalInput").ap()
    d_vw = nc.dram_tensor("vw", [PPC, 128, 63 * 65], bf16, kind="ExternalInput").ap()
    d_vr = nc.dram_tensor("vr", [PPC, 128, NMID * 2 * 65], bf16, kind="ExternalInput").ap()
    d_vg = nc.dram_tensor("vg", [PPC, 128, 65], bf16, kind="ExternalInput").ap()
    d_vg1 = nc.dram_tensor("vg1", [PPC, 128, 65], bf16, kind="ExternalInput").ap()
    d_vg62 = nc.dram_tensor("vg62", [PPC, 128, 65], bf16, kind="ExternalInput").ap()
    d_out = nc.dram_tensor("out", [PPC, 65, S], bf16, kind="ExternalOutput").ap()

    with TileContext(nc) as tc:
        with tc.tile_pool(name="sb", bufs=2) as sb, \
             tc.tile_pool(name="ps", bufs=2, space="PSUM") as ps, \
             tc.tile_pool(name="ptp", bufs=4) as ptp, \
             tc.tile_pool(name="aux", bufs=2) as aux:

            for p in range(PPC):
                # K-side and Q-side tiles are 128 partitions tall: rows 0:64
                # carry data (DMA), rows 64:128 are zeroed so every matmul
                # contracts K=128 (the tensor engine only clocks up under
                # full-partition contractions).
                qt = sb.tile([128, S], bf16, name=f"qt{p}", tag="qt")
                kt = sb.tile([128, S], bf16, name=f"kt{p}", tag="kt")
                qtd = sb.tile([128, 128], bf16, name=f"qtd{p}", tag="qtd")
                ktg = sb.tile([128, 128], bf16, name=f"ktg{p}", tag="ktg")
                kr = sb.tile([128, NMID * 256], bf16, name=f"kr{p}", tag="kr")
                vw = sb.tile([128, 63 * 65], bf16, name=f"vw{p}", tag="vw")
                vr = sb.tile([128, NMID * 2 * 65], bf16, name=f"vr{p}", tag="vr")
                vg = sb.tile([128, 65], bf16, name=f"vg{p}", tag="vg")
                vg1 = sb.tile([128, 65], bf16, name=f"vg1{p}", tag="vg1")
                vg62 = sb.tile([128, 65], bf16, name=f"vg62{p}", tag="vg62")

                # scalar queue: only what the dense phase needs first
                nc.scalar.dma_start(out=qtd[0:64, :], in_=d_qtd[p])
                nc.scalar.dma_start(out=kt[0:64, :], in_=d_kt[p])
                nc.sync.dma_start(out=ktg[0:64, :], in_=d_ktg[p])
                for t_, d_ in ((vg, d_vg), (vg1, d_vg1), (vg62, d_vg62)):
                    nc.sync.dma_start(out=t_, in_=d_[p])
                nc.sync.dma_start(out=qt[0:64, :], in_=d_qt[p])
                nc.sync.dma_start(out=vw, in_=d_vw[p])
                h_kr = NMID * 128
                nc.gpsimd.dma_start(out=kr[0:64, 0:h_kr], in_=d_kr[p][:, 0:h_kr])
                nc.gpsimd.dma_start(out=kr[0:64, h_kr:], in_=d_kr[p][:, h_kr:])
                h_vr = NMID * 65
                nc.gpsimd.dma_start(out=vr[:, 0:h_vr], in_=d_vr[p][:, 0:h_vr])
                nc.gpsimd.dma_start(out=vr[:, h_vr:], in_=d_vr[p][:, h_vr:])

                # zero the stacked contraction rows once per pool buffer
                # (pairs >= 2 reuse the buffers; DMA never dirties rows
                # 64:128, so the zeros persist)
                if p < 2:
                    nc.vector.memset(qtd[64:128, :], 0.0)
                    nc.vector.memset(ktg[64:128, :], 0.0)
                    nc.vector.memset(kt[64:128, :], 0.0)
                    nc.vector.memset(qt[64:128, :], 0.0)
                    h2 = NMID * 128
                    nc.vector.memset(kr[64:128, 0:h2], 0.0)
                    nc.gpsimd.memset(kr[64:128, h2:], 0.0)

                ostage = aux.tile([65, S], bf16, name=f"ostage{p}", tag="os")

                def vw_pack(j):                     # keys j*64 .. j*64+128
                    return vw[:, j * 65:(j + 1) * 65]

                # dense stage c: 8 key packs (even j = 16c..16c+14)
                def dense_qk(c):
                    st = ps.tile([128, 1024], f32, name=f"std{p}_{c}", tag="st",
                                 bufs=3)
                    for j in range(8):
                        nc.tensor.matmul(
                            st[:, j * 128:(j + 1) * 128],
                            lhsT=kt[:, (8 * c + j) * 128:(8 * c + j + 1) * 128],
                            rhs=qtd, start=True, stop=True)
                    pt = ptp.tile([128, 1024], bf16, name=f"ptd{p}_{c}", tag="pt")
                    nc.scalar.activation(pt, st, EXP, scale=SCALE, bias=EBIAS)
                    return st, pt

                def dense_pv(c, st_pt, ctxd):
                    st, pt = st_pt
                    for j in range(8):
                        nc.tensor.matmul(
                            ctxd[0:65, 0:128],
                            lhsT=vw_pack(2 * (8 * c + j)),
                            rhs=pt[:, j * 128:(j + 1) * 128],
                            start=(c == 0 and j == 0), stop=(c == 3 and j == 7))

                # middle group g: nl blocks starting at l0
                def group_qk(g):
                    l0, nl = GROUPS[g]
                    st = ps.tile([128, 1024], f32, name=f"st{p}_{g}", tag="st",
                                 bufs=3)
                    for j in range(nl):
                        l = l0 + j
                        o = j * 256
                        rhs = qt[:, l * BLK:(l + 1) * BLK]
                        nc.tensor.matmul(st[:, o:o + 64],
                                         lhsT=kt[:, (l - 1) * BLK:(l + 1) * BLK],
                                         rhs=rhs, start=True, stop=True)
                        kb_ = kr[:, (l - 1) * 256:(l - 1) * 256 + 128]
                        kc_ = kr[:, (l - 1) * 256 + 128:(l - 1) * 256 + 256]
                        nc.tensor.matmul(st[:, o + 64:o + 128], lhsT=kb_,
                                         rhs=rhs, start=True, stop=True)
                        nc.tensor.matmul(st[:, o + 128:o + 192], lhsT=kc_,
                                         rhs=rhs, start=True, stop=True)
                        nc.tensor.matmul(st[:, o + 192:o + 256], lhsT=ktg,
                                         rhs=rhs, start=True, stop=True)
                    pt = ptp.tile([128, 1024], bf16, name=f"pt{p}_{g}", tag="pt")
                    nc.scalar.activation(pt[:, 0:nl * 256], st[:, 0:nl * 256],
                                         EXP, scale=SCALE, bias=EBIAS)
                    return st, pt

                def group_pv(g, st_pt, ctx):
                    st, pt = st_pt
                    l0, nl = GROUPS[g]
                    for j in range(nl):
                        l = l0 + j
                        o = j * 256
                        oc = j * BLK
                        vb_ = vr[:, (l - 1) * 130:(l - 1) * 130 + 65]
                        vc_ = vr[:, (l - 1) * 130 + 65:(l - 1) * 130 + 130]
                        vg_ = vg1 if l == 1 else (vg62 if l == 62 else vg)
                        nc.tensor.matmul(ctx[0:65, oc:oc + 64],
                                         lhsT=vw_pack(l - 1),
                                         rhs=pt[:, o:o + 64],
                                         start=True, stop=False)
                        nc.tensor.matmul(ctx[0:65, oc:oc + 64], lhsT=vb_,
                                         rhs=pt[:, o + 64:o + 128],
                                         start=False, stop=False)
                        nc.tensor.matmul(ctx[0:65, oc:oc + 64], lhsT=vc_,
                                         rhs=pt[:, o + 128:o + 192],
                                         start=False, stop=False)
                        nc.tensor.matmul(ctx[0:65, oc:oc + 64], lhsT=vg_,
                                         rhs=pt[:, o + 192:o + 256],
                                         start=False, stop=True)

                def group_out(g, ctx):
                    l0, nl = GROUPS[g]
                    w = nl * BLK
                    nc.vector.tensor_copy(ostage[:, l0 * BLK:l0 * BLK + w],
                                          ctx[0:65, 0:w])

                # ---- emit: dense chunks + groups, depth-2 pipelined ----
                ctxd = ps.tile([128, 128], f32, name=f"ctxd{p}", tag="ctx")
                stages = [("d", c) for c in range(4)] + \
                         [("g", g) for g in range(len(GROUPS))]

                def run_qk(s):
                    kind, i = s
                    if kind == "g":
                        ctx_of[i] = ps.tile([128, 256], f32,
                                            name=f"ctx{p}_{i}", tag="ctx")
                    return dense_qk(i) if kind == "d" else group_qk(i)

                def run_pv(s, st_pt):
                    kind, i = s
                    if kind == "d":
                        dense_pv(i, st_pt, ctxd)
                        if i == 3:
                            nc.vector.tensor_copy(ostage[:, 0:BLK],
                                                  ctxd[0:65, 0:64])
                            nc.vector.tensor_copy(ostage[:, S - BLK:],
                                                  ctxd[0:65, 64:128])
                    else:
                        group_pv(i, st_pt, ctx_of[i])
                        group_out(i, ctx_of[i])

                ctx_of = {}
                pend = []
                for s in stages:
                    pend.append((s, run_qk(s)))
                    if len(pend) > 2:
                        ps_, st_ = pend.pop(0)
                        run_pv(ps_, st_)
                for ps_, st_ in pend:
                    run_pv(ps_, st_)

                nc.sync.dma_start(out=d_out[p], in_=ostage)

    if apply_fixup:
        _fixup_multiwait(nc, mybir)
    return nc


def _get_program():
    if "nc" not in _COMPILED:
        _COMPILED["nc"] = _build_program()
    return _COMPILED["nc"]


def kernel(query_layer, key_layer, value_layer, band_mask, from_mask, to_mask,
           from_blocked_mask, to_blocked_mask, rand_attn):
    import sys
    if "/opt/trn_rl_repo" not in sys.path:
        sys.path.insert(0, "/opt/trn_rl_repo")
    from concourse.bass_utils import run_bass_kernel_spmd

    arrs = _build_host_arrays(query_layer, key_layer, value_layer, rand_attn)
    nc = _get_program()

    in_maps = []
    for c in range(NCORE):
        sl = slice(c * PPC, (c + 1) * PPC)
        in_maps.append({k: np.ascontiguousarray(v[sl]) for k, v in arrs.items()})

    res = run_bass_kernel_spmd(nc, in_maps, list(range(NCORE)))

    outs = np.stack([np.asarray(res.results[c]["out"]) for c in range(NCORE)])
    outs = outs.reshape(NPAIR, 65, S).astype(np.float64)
    ctx = outs[:, :64, :] / outs[:, 64:65, :]                        # [24,64,S]
    ctx = ctx.transpose(0, 2, 1).reshape(B, H, S, D)                 # [B,H,S,D]
    out = ctx.transpose(0, 2, 1, 3).astype(np.float32)               # [B,S,H,D]
    return np.ascontiguousarray(out)
